# revision 10
# baseline (speedup 1.0000x reference)
"""Trainium2 Bass kernel for nn_FastAttention: out = v + q @ (k^T @ v) per (b,h).

Full shapes: q,k,v [B=2, H=16, S=4096, D=128] f32.
Sharding: B*H = 32 pairs split across 8 cores -> 4 pairs/core, no collectives.

Per (b,h) pair on-core:
  phase A: kv[d,e] = sum_s k[s,d] v[s,e]    (32 accumulating fp32 matmuls)
  phase T: qT[d,s] = q[s,d]^T               (PE transpose via identity, fp32)
  phase B: out[s,e] = v[s,e] + sum_d qT[d,s] kv[d,e]   (bf16 matmuls)

The kernel is DMA-bound: 32MB/core of HBM traffic at the measured
~420GB/s/core saturates from ~8us (fixed preamble) to ~84us; the graded
time is that window plus the tail chain after the last input byte plus a
~3us fixed epilogue.  Schedule principles:
  - SBUF layout tile[p, n*128+d] = x[32p+n, d]: every load/store is 4-8KB
    contiguous per partition (line rate); a matmul "chunk" is the strided
    row-set {32p+j}, a plain column slice of the tile.
  - Phase B runs in bf16: qT and kv are rounded to bf16 for free inside
    the PSUM->SBUF copies that already existed (ACT copy after the
    transposes; DVE copy after phase A).  bf16 streams 1 cycle/row vs
    fp32's 4, cutting PE active time ~1.4x and PE power (fp32 runs 2
    passes/row) -- PE power-throttling stretched compute on straggler
    cores until it gated the DMA stream.  k^T v stays exact fp32;
    measured rel err ~3e-3 (threshold 2e-2).
  - k,v load in quarters so phase A (chunk n needs only chunk n) tracks
    the arriving data instead of serializing after it.
  - The tail after the last input byte is the whole game.  The last
    ring entries crawl (the DGE services a ring's final descriptors at
    single-engine rate), so whatever loads last must gate minimal work:
      * the LAST pair's q loads move to the otherwise-idle SWDGE ring
        mid-schedule (issued while pair 2 computes), and its transposes
        are emitted BEFORE its phase A so the in-order PE runs them as
        soon as q lands rather than after A;
      * the last input bytes are then pair 3's k,v quarters, which gate
        only the A-tail plus the bf16 B chain (~4us), not a transpose
        pipeline;
      * pair 3's stores ride the sync ring, whose sequencer is idle
        once its final k,v quarters have issued, so store issue
        pipelines with the B chain instead of queueing behind loads.
  - k,v triple-buffered so pair p+1's loads never wait on compute;
    q/qT/o double-buffered.
  - For pairs 0-2, T(g+1) is emitted before B(g) so the PSUM->SBUF qT
    copy (ACT) hides behind the next group's transposes; qT copies all
    on ACT and v-adds all on DVE so neither in-order queue blocks the
    other.
"""

import sys

if "/opt/trn_rl_repo" not in sys.path:
    sys.path.insert(0, "/opt/trn_rl_repo")

import numpy as np

import concourse.bass as bass
import concourse.mybir as mybir
import concourse.tile as tile
from concourse import bacc
from concourse.bass import ds, ts
from concourse.bass_utils import run_bass_kernel_spmd
from concourse.masks import make_identity

B, H, S, D = 2, 16, 4096, 128
N_CORES = 8
PAIRS = (B * H) // N_CORES  # 4
F32 = mybir.dt.float32
BF16 = mybir.dt.bfloat16


def build_nc(pairs=PAIRS, s=S):
    nc = bacc.Bacc(
        "TRN2", target_bir_lowering=False, debug=False, num_devices=N_CORES
    )
    q = nc.dram_tensor("q", [pairs, s, D], F32, kind="ExternalInput").ap()
    k = nc.dram_tensor("k", [pairs, s, D], F32, kind="ExternalInput").ap()
    v = nc.dram_tensor("v", [pairs, s, D], F32, kind="ExternalInput").ap()
    out = nc.dram_tensor("out", [pairs, s, D], F32, kind="ExternalOutput").ap()

    nch = s // 128  # s-chunks per pair
    gsz = 4  # chunks per psum group (512 free-dim)
    ngrp = nch // gsz

    with tile.TileContext(nc) as tc:
        with (
            tc.tile_pool(name="const", bufs=1) as cpool,
            tc.tile_pool(name="kvio", bufs=3) as kvio,
            tc.tile_pool(name="qio", bufs=2) as qio,
            tc.tile_pool(name="pskv", bufs=2, space="PSUM") as pskv,
            tc.tile_pool(name="psq", bufs=3, space="PSUM") as psq,
            tc.tile_pool(name="pso", bufs=3, space="PSUM") as pso,
        ):
            ident = cpool.tile([128, 128], F32)
            make_identity(nc, ident[:])

            q3_tile = [None]

            def emit_q_loads(p, q_sb, engine, spans):
                q3h = q[p].rearrange("(p n) d -> p n d", p=128)
                q_t3 = q_sb[:].rearrange("p (n d) -> p n d", d=128)
                for st, ln in spans:
                    hs = ds(st, ln)
                    engine.dma_start(out=q_t3[:, hs], in_=q3h[:, hs])

            for p in range(pairs):
                last = p == pairs - 1
                k_sb = kvio.tile([128, s], F32, tag="k")
                v_sb = kvio.tile([128, s], F32, tag="v")
                if last:
                    q_sb = q3_tile[0]
                else:
                    q_sb = qio.tile([128, s], F32, tag="q")
                qT_sb = qio.tile([128, s], BF16, tag="qT")
                o_sb = qio.tile([128, s], F32, tag="o")
                kv_sb = qio.tile([128, 128], BF16, tag="kv")

                # loads: partition p holds rows 32p..32p+31 (16KB contiguous);
                # chunk j is the strided row-set {32p+j}.  k,v (and pairs
                # 0-2's q) on the SP HWDGE ring; issuing from nc.scalar would
                # head-of-line block behind the qT copies on the ACT
                # sequencer (measured +15us).
                k3 = k[p].rearrange("(p n) d -> p n d", p=128)
                v3 = v[p].rearrange("(p n) d -> p n d", p=128)
                k_t3 = k_sb[:].rearrange("p (n d) -> p n d", d=128)
                v_t3 = v_sb[:].rearrange("p (n d) -> p n d", d=128)
                qtr = nch // 4
                for h in range(4):
                    hs = ts(h, qtr)
                    nc.sync.dma_start(out=k_t3[:, hs], in_=k3[:, hs])
                    nc.sync.dma_start(out=v_t3[:, hs], in_=v3[:, hs])
                if not last:
                    emit_q_loads(
                        p, q_sb, nc.sync, [(0, 8), (8, 8), (16, 8), (24, 8)]
                    )
                if p == pairs - 2:
                    # pre-issue the LAST pair's q on the SWDGE ring: its
                    # instructions sit after stores(p-1) in that queue, so
                    # they issue while this pair computes and land well
                    # before pair 3's k,v finish on the sync ring.
                    q3_tile[0] = qio.tile([128, s], F32, tag="q", name="q3_sb")
                    emit_q_loads(
                        pairs - 1,
                        q3_tile[0],
                        nc.gpsimd,
                        [(0, 8), (8, 8), (16, 8), (24, 8)],
                    )

                o3 = out[p].rearrange("(p n) d -> p n d", p=128)
                o_t3 = o_sb[:].rearrange("p (n d) -> p n d", d=128)

                def emit_T(g):
                    qt_ps = psq.tile([128, gsz * 128], F32, tag="qt_ps")
                    for j in range(gsz):
                        n = g * gsz + j
                        nc.tensor.transpose(
                            qt_ps[:, ts(j, 128)], q_sb[:, ts(n, 128)], ident[:]
                        )
                    # ACT, not DVE: keeps the copy off DVE's in-order queue
                    # (which carries the v-adds); the copy also rounds the
                    # fp32 transpose result to bf16 for phase B.
                    nc.scalar.copy(qT_sb[:, ts(g, gsz * 128)], qt_ps[:])

                def emit_A():
                    kv_ps = pskv.tile([128, 128], F32, tag="kv_ps")
                    for n in range(nch):
                        nc.tensor.matmul(
                            kv_ps[:],
                            lhsT=k_sb[:, ts(n, 128)],
                            rhs=v_sb[:, ts(n, 128)],
                            start=(n == 0),
                            stop=(n == nch - 1),
                        )
                    nc.vector.tensor_copy(kv_sb[:], kv_ps[:])

                def emit_B(g):
                    o_ps = pso.tile([128, gsz * 128], F32, tag="o_ps")
                    for j in range(gsz):
                        n = g * gsz + j
                        nc.tensor.matmul(
                            o_ps[:, ts(j, 128)],
                            lhsT=qT_sb[:, ts(n, 128)],
                            rhs=kv_sb[:],
                            start=True,
                            stop=True,
                        )
                    nc.vector.tensor_add(
                        o_sb[:, ts(g, gsz * 128)],
                        o_ps[:],
                        v_sb[:, ts(g, gsz * 128)],
                    )

                if last:
                    # q is already resident: transpose it ALL first (the
                    # in-order PE would otherwise hold T behind A while A
                    # waits for the final v quarters), then run A tracking
                    # the arriving k,v, then the whole bf16 B chain.
                    for g in range(ngrp):
                        emit_T(g)
                    emit_A()
                    stored = 0
                    for g in range(ngrp):
                        emit_B(g)
                        done = (g + 1) * gsz
                        # stores on sync (idle after its last k,v quarter);
                        # 8-chunk granules, 4-chunk for the final two so the
                        # last transfer is small.
                        if done % 8 == 0 or done > 24:
                            hs = ds(stored, done - stored)
                            nc.sync.dma_start(out=o3[:, hs], in_=o_t3[:, hs])
                            stored = done
                else:
                    emit_A()
                    emit_T(0)
                    stored = 0
                    for g in range(ngrp):
                        if g + 1 < ngrp:
                            emit_T(g + 1)
                        emit_B(g)
                        done = (g + 1) * gsz
                        if done % 8 == 0:
                            hs = ds(stored, done - stored)
                            nc.gpsimd.dma_start(out=o3[:, hs], in_=o_t3[:, hs])
                            stored = done
    nc.finalize()
    return nc


def kernel(q, k, v, _trace=False):
    q = np.ascontiguousarray(np.asarray(q, dtype=np.float32)).reshape(B * H, S, D)
    k = np.ascontiguousarray(np.asarray(k, dtype=np.float32)).reshape(B * H, S, D)
    v = np.ascontiguousarray(np.asarray(v, dtype=np.float32)).reshape(B * H, S, D)

    nc = build_nc()
    in_maps = [
        {
            "q": q[i * PAIRS : (i + 1) * PAIRS],
            "k": k[i * PAIRS : (i + 1) * PAIRS],
            "v": v[i * PAIRS : (i + 1) * PAIRS],
        }
        for i in range(N_CORES)
    ]
    res = run_bass_kernel_spmd(nc, in_maps, core_ids=list(range(N_CORES)))
    full = np.concatenate([res.results[i]["out"] for i in range(N_CORES)], axis=0)
    out = full.reshape(B, H, S, D)
    if _trace:
        # repeat traced executes: the executable is compiled+cached after the
        # first run, so each NTFF profile context wraps only an execute.
        # Multiple samples filter out co-tenant HBM-contention noise.
        tres = [
            run_bass_kernel_spmd(
                nc,
                in_maps,
                core_ids=list(range(N_CORES)),
                trace=True,
                trace_cores=list(range(N_CORES)),
            )
            for _ in range(3)
        ]
        return out, tres
    return out
